# revision 28
# baseline (speedup 1.0000x reference)
"""Trainium2 Bass kernel for nn_CrossAttentionFusion.

Sharding: 8 cores = (batch b in 0..4) x (image-row half in 0..2).
Each core computes cross-attention for its 2048 query pixels (K/V over the
full 4096 pixels of its batch element, K/V compute replicated within the
pair), then the conv+BN+relu stack for its half of the image.

V2 structure:
  - projections are fused into the attention stream (no separate PSUM
    phase): k1/q2 projections start as soon as their first feature-map
    DMA pieces land (~16us), attention chunk (1,0) follows immediately,
    and the remaining projections (q1, k2, V1, V2, rest of q2) are
    injected between attention m-tiles so the PE never drains.
  - DMAs are issued in consumption order (w_k1/w_q2/w_v1 first, then
    interleaved F1/F2 column pieces, conv weights last).
  - softmax reciprocal on Scalar (ln+exp) instead of the DVE (3.3us).
  - halo rows (attention X1 ghosts + conv ghosts) exchanged with
    PAIRWISE AllGathers; BN statistics with 8-rank AllReduce of
    (sum, sumsq) so no gather/reduce is needed on the receive side.
  - conv halo AG fires when chunks 0/3 of the last cit close (close
    order 0,3,2,1), half a cit before the stats AllReduce.
  - conv1 consumes the a=1 channels (X1 cits 2,3 -- finished early in
    the chunk order) first, so its first psum groups can start while
    the last attention chunks are still streaming.
"""

import os
import numpy as np
import ml_dtypes

N_CORES = 8
B, C, H, W = 4, 256, 64, 64
HW = H * W              # 4096
NQ = 2048               # queries per core
CH = 512                # free-dim chunk (1 PSUM bank of f32)
NCH = NQ // CH          # 4
MT = HW // 128          # 32 m-tiles
EPS = 1e-5
PADW = 66
PADR = 34
NPAD = PADR * PADW      # 2244

# conv layer specs: (Cin, Cout, cin_tiles, cout_tiles)
CONVS = [(512, 256, 4, 2), (256, 128, 2, 1), (128, 64, 1, 1)]

_CACHE = {}


# --------------------------------------------------------------------------
# walrus in this container rejects >1 sync wait per instruction; split extras
# onto same-engine NOPs placed immediately before the offending instruction.
def _split_excess_waits(nc, max_waits=1):
    import bass_rust
    eng_map = dict(nc.engines)
    n_split = 0
    for fn in nc.m.functions:
        for bb in fn.blocks:
            lst = bb.instructions
            i = 0
            while i < len(lst):
                inst = lst[i]
                si = inst.sync_info
                if si is not None and len(si.on_wait) > max_waits:
                    waits = list(si.on_wait)
                    keep = waits[-max_waits:]
                    extra = waits[:-max_waits]
                    inst.sync_info = bass_rust.SyncInfo(
                        on_wait=keep, on_update=list(si.on_update))
                    eng = eng_map[inst.engine]
                    pos = i
                    for j in range(0, len(extra), max_waits):
                        chunk = extra[j:j + max_waits]
                        bi = eng.nop()
                        nop_inst = bi.ins
                        cur = nc.cur_bb.bb.instructions
                        assert cur[-1] is nop_inst
                        cur.pop()
                        nop_inst.sync_info = bass_rust.SyncInfo(
                            on_wait=chunk, on_update=[])
                        lst.insert(pos, nop_inst)
                        pos += 1
                        i += 1
                    n_split += 1
                i += 1
    return n_split


def _build_program(debug=False):
    import concourse.bass as bass
    import concourse.mybir as mybir
    import concourse.tile as tile

    f32 = mybir.dt.float32
    bf16 = mybir.dt.bfloat16
    FT = mybir.ActivationFunctionType

    nc = bass.Bass()

    # ---------------- DRAM I/O (all host pre-shuffled, contiguous) --------
    d_f1 = nc.dram_tensor("f1", [128, 2 * HW], bf16, kind="ExternalInput")
    d_f2 = nc.dram_tensor("f2", [128, 2 * HW], bf16, kind="ExternalInput")
    d_w = {}
    d_b = {}
    for nm in ("q1", "k2", "v2", "q2", "k1", "v1"):
        d_w[nm] = nc.dram_tensor(f"w_{nm}", [128, 2 * C], bf16,
                                 kind="ExternalInput")
    for nm in ("q1", "k2", "q2", "k1"):
        d_b[nm] = nc.dram_tensor(f"b_{nm}", [128, 2], f32,
                                 kind="ExternalInput")
    d_bv1r = nc.dram_tensor("bv1r", [128, C], f32, kind="ExternalInput")
    d_bv2r = nc.dram_tensor("bv2r", [128, C], f32, kind="ExternalInput")

    d_wc, d_bn = [], []
    for li, (cin, cout, cit_n, cot_n) in enumerate(CONVS):
        d_wc.append(nc.dram_tensor(f"wc{li}", [128, cit_n * 9 * cout], bf16,
                                   kind="ExternalInput"))
        parts = min(cout, 128)
        # bc, gamma, beta stacked: [parts, 3*cot_n]
        d_bn.append(nc.dram_tensor(f"bn{li}", [parts, 3 * cot_n], f32,
                                   kind="ExternalInput"))

    d_sel2 = nc.dram_tensor("sel2", [128, 2, 64, 2], bf16, kind="ExternalInput")
    d_self = nc.dram_tensor("selF2", [128, 2, 64, 2], f32, kind="ExternalInput")
    d_rowm = nc.dram_tensor("rowm", [128, 2, 64], bf16, kind="ExternalInput")

    d_out = nc.dram_tensor("yout", [64, 32 * 64], f32, kind="ExternalOutput")

    # collective buffers
    replica8 = [list(range(N_CORES))]
    rpairs = [[2 * i, 2 * i + 1] for i in range(N_CORES // 2)]
    # attention halo rows: pairwise AG, one per a-half (2 cits x 2 rows x 64)
    cc0i = [nc.dram_tensor(f"cc0i{g}", [128, 256], bf16) for g in range(2)]
    cc0o = [nc.dram_tensor(f"cc0o{g}", [2, 128, 256], bf16) for g in range(2)]
    # conv BN stats: 8-rank AllReduce of (sum, sumsq)
    sli, slo = [], []
    # conv halo rows: pairwise AG of (row1, row32) f32
    hli, hlo = [], []
    for li, (cin, cout, cit_n, cot_n) in enumerate(CONVS):
        parts = min(cout, 128)
        sli.append([nc.dram_tensor(f"s{li}i{c_}", [parts, 2], f32)
                    for c_ in range(cot_n)])
        slo.append([nc.dram_tensor(f"s{li}o{c_}", [8, parts, 2], f32,
                                   addr_space="Shared")
                    for c_ in range(cot_n)])
        if li < 2:
            hli.append([nc.dram_tensor(f"h{li}i{c_}", [parts, 128], f32)
                        for c_ in range(cot_n)])
            hlo.append([nc.dram_tensor(f"h{li}o{c_}", [2, parts, 128], f32)
                        for c_ in range(cot_n)])

    dbg = {}
    if debug:
        dbg["qT1"] = nc.dram_tensor("d_qT1", [128, 2, NQ], bf16, kind="ExternalOutput")
        dbg["kT2"] = nc.dram_tensor("d_kT2", [128, 2, HW], bf16, kind="ExternalOutput")
        dbg["V2"] = nc.dram_tensor("d_V2", [128, MT, C], bf16, kind="ExternalOutput")
        dbg["Z"] = nc.dram_tensor("d_Z", [2, NCH, CH], f32, kind="ExternalOutput")
        dbg["X1"] = nc.dram_tensor("d_X1", [128, 4, NPAD], bf16, kind="ExternalOutput")
        dbg["Y1"] = nc.dram_tensor("d_Y1", [2, 128, NQ], f32, kind="ExternalOutput")
        dbg["SC1"] = nc.dram_tensor("d_SC1", [2, 128, 2], f32, kind="ExternalOutput")
        dbg["X2"] = nc.dram_tensor("d_X2", [128, 2, NPAD], bf16, kind="ExternalOutput")

    with tile.TileContext(nc) as tc:
        with (
            tc.tile_pool(name="consts", bufs=1) as consts,
            tc.tile_pool(name="xpads", bufs=1) as xpads,
            tc.tile_pool(name="stage", bufs=2) as stage,
        ):
            # qkv pool outlives fmaps (LIFO pool release)
            qkv_ctx = tc.tile_pool(name="qkv", bufs=1)
            qkv = qkv_ctx.__enter__()
            qT = [qkv.tile([128, 2, NQ], bf16, name="qT1", tag="qT1"),
                  qkv.tile([128, 2, NQ], bf16, name="qT2", tag="qT2")]
            kT = [qkv.tile([128, 2, HW], bf16, name="kT2", tag="kT2"),
                  qkv.tile([128, 2, HW], bf16, name="kT1", tag="kT1")]
            Vv = [qkv.tile([128, MT, C], bf16, name="V2", tag="V2"),
                  qkv.tile([128, MT, C], bf16, name="V1", tag="V1")]

            fmaps_ctx = tc.tile_pool(name="fmaps", bufs=1)
            fmaps = fmaps_ctx.__enter__()
            F1 = fmaps.tile([128, 2, HW], bf16, name="F1")
            F2 = fmaps.tile([128, 2, HW], bf16, name="F2")

            # ---- const tiles (allocate first, DMA in priority order) ----
            w_sb = {}
            for nm in ("q1", "k2", "v2", "q2", "k1", "v1"):
                w_sb[nm] = consts.tile([128, 2, C], bf16, name=f"w_{nm}",
                                       tag=f"w_{nm}")
            b_sb = {}
            for nm in ("q1", "k2", "q2", "k1"):
                b_sb[nm] = consts.tile([128, 2], f32, name=f"b_{nm}",
                                       tag=f"b_{nm}")
            bv1r = consts.tile([128, C], f32, name="bv1r")
            bv2r = consts.tile([128, C], f32, name="bv2r")
            sel2 = consts.tile([128, 2, 64, 2], bf16, name="sel2")
            selF2 = consts.tile([128, 2, 64, 2], f32, name="selF2")
            rowm = consts.tile([128, 2, 64], bf16, name="rowm")
            wc_sb = []
            bn_sb = []
            for li, (cin, cout, cit_n, cot_n) in enumerate(CONVS):
                parts = min(cout, 128)
                wc_sb.append(consts.tile([128, cit_n, 3, 3, cout], bf16,
                                         name=f"wc{li}", tag=f"wc{li}"))
                bn_sb.append(consts.tile([parts, 3, cot_n], f32,
                                         name=f"bn{li}", tag=f"bn{li}"))

            # ---- DMA kicks in consumption order ------------------------
            # NEVER kick from Scalar/Vector/Tensor: a SWDGE dma_start costs
            # the issuing engine ~0.7us and the projection evictions / exp
            # stream sit behind it in program order (measured: Scalar was
            # blocked until 35us when it carried kicks).
            dmae = [nc.sync, nc.gpsimd]
            _ki = [0]

            def kick(out, in_):
                dmae[_ki[0] % 2].dma_start(out=out, in_=in_)
                _ki[0] += 1

            PC = 1024       # fmap DMA piece: [128, 1, 1024] = 2KB/partition

            def fpiece(Ft, dt, c, kt):
                kick(Ft[:, kt, c * PC:(c + 1) * PC],
                     dt[:, kt * HW + c * PC:kt * HW + (c + 1) * PC])

            # first-needed: k1 (F1 c0), q2 ch0 (F2 c0), v1 (F1)
            kick(w_sb["k1"][:], d_w["k1"][:])
            kick(b_sb["k1"][:], d_b["k1"][:])
            def fhalf(Ft, dt, lo, kt):
                kick(Ft[:, kt, lo:lo + 512],
                     dt[:, kt * HW + lo:kt * HW + lo + 512])

            fhalf(F1, d_f1, 0, 0)
            fhalf(F1, d_f1, 0, 1)
            kick(w_sb["q2"][:], d_w["q2"][:])
            kick(b_sb["q2"][:], d_b["q2"][:])
            fhalf(F2, d_f2, 0, 0)
            fhalf(F2, d_f2, 0, 1)
            fhalf(F1, d_f1, 512, 0)
            fhalf(F1, d_f1, 512, 1)
            fhalf(F2, d_f2, 512, 0)
            fhalf(F2, d_f2, 512, 1)
            kick(w_sb["v1"][:], d_w["v1"][:])
            kick(bv1r[:], d_bv1r[:])
            for c in range(1, 4):
                fpiece(F1, d_f1, c, 0)
                fpiece(F1, d_f1, c, 1)
            kick(w_sb["q1"][:], d_w["q1"][:])
            kick(b_sb["q1"][:], d_b["q1"][:])
            kick(w_sb["k2"][:], d_w["k2"][:])
            kick(b_sb["k2"][:], d_b["k2"][:])
            kick(w_sb["v2"][:], d_w["v2"][:])
            kick(bv2r[:], d_bv2r[:])
            for c in range(1, 4):
                fpiece(F2, d_f2, c, 0)
                fpiece(F2, d_f2, c, 1)
            kick(sel2[:], d_sel2[:])
            kick(selF2[:], d_self[:])
            kick(rowm[:], d_rowm[:])
            for li in range(len(CONVS)):
                kick(bn_sb[li][:], d_bn[li][:])
            for li in range(len(CONVS)):
                kick(wc_sb[li][:], d_wc[li][:])

            # ---- constants ----
            ones_b = consts.tile([128, 1], bf16, name="ones_b")
            nc.vector.memset(ones_b, 1.0)
            ones1b = consts.tile([1, 128], bf16, name="ones1b")
            nc.vector.memset(ones1b, 1.0)
            epsb = consts.tile([128, 1], f32, name="epsb")
            nc.vector.memset(epsb, EPS)

            # ---- persistent activations ----
            X1 = xpads.tile([128, 4, NPAD], bf16, name="X1")
            nc.vector.memset(X1, 0.0)

            # =========== fused projections + attention ===========
            with (
                tc.tile_pool(name="stps", bufs=3, space="PSUM") as stps,
                tc.tile_pool(name="pvps", bufs=4, space="PSUM") as pvps,
                tc.tile_pool(name="zbz", bufs=1, space="PSUM") as zbz,
                tc.tile_pool(name="attnw", bufs=2) as attnw,
                tc.tile_pool(name="epool", bufs=4) as epool,
                tc.tile_pool(name="ghw", bufs=2) as ghw,
            ):
                def proj_qk(dst, wname, Fsrc, ct, ch):
                    ps = stps.tile([128, CH], f32, name="pps", tag="st")
                    wt = w_sb[wname]
                    for kt in range(2):
                        nc.tensor.matmul(
                            ps[:], wt[:, kt, ct * 128:(ct + 1) * 128],
                            Fsrc[:, kt, ch * CH:(ch + 1) * CH],
                            start=(kt == 0), stop=(kt == 1))
                    nc.scalar.activation(
                        out=dst[:, ct, ch * CH:(ch + 1) * CH], in_=ps[:],
                        func=FT.Identity, bias=b_sb[wname][:, ct:ct + 1])

                def proj_qk_pack(dst, wname, Fsrc, ch):
                    for ct in range(2):
                        proj_qk(dst, wname, Fsrc, ct, ch)

                def proj_v_mt(a, mt):
                    dst, wname, Fsrc, bvr = (
                        (Vv[1], "v1", F1, bv1r) if a == 1
                        else (Vv[0], "v2", F2, bv2r))
                    ps = stps.tile([128, CH], f32, name="pps", tag="st")
                    wt = w_sb[wname]
                    for kt in range(2):
                        nc.tensor.matmul(
                            ps[:, 0:C], Fsrc[:, kt, mt * 128:(mt + 1) * 128],
                            wt[:, kt, :], start=(kt == 0), stop=(kt == 1))
                    nc.vector.tensor_add(dst[:, mt, :], ps[:, 0:C], bvr[:])

                def attn_chunk(a, ch, pv, esum, inject=None, tail=None):
                    """Emit one attention chunk; inject[mt] is a list of
                    hooks emitted before m-tile mt's score matmuls (used to
                    software-pipeline the previous chunk's epilogue and the
                    remaining projections into this chunk's score stream).
                    tail() (if given) is emitted between the mt loop and the
                    final zpv flush, so the flush matmuls cover its latency."""
                    inject = dict(inject or {})
                    Q, K, V = qT[a], kT[a], Vv[a]

                    def zpv(pmt, e):
                        for ct in range(2):
                            nc.tensor.matmul(
                                pv[ct][:],
                                V[:, pmt, ct * 128:(ct + 1) * 128], e[:],
                                start=(pmt == 0), stop=(pmt == MT - 1))

                    pend = []
                    for mt in range(MT):
                        for fn in inject.pop(mt, ()):
                            fn()
                        st = stps.tile([128, CH], f32, name="st", tag="st")
                        for kt in range(2):
                            nc.tensor.matmul(
                                st[:], K[:, kt, mt * 128:(mt + 1) * 128],
                                Q[:, kt, ch * CH:(ch + 1) * CH],
                                start=(kt == 0), stop=(kt == 1))
                        e = epool.tile([128, CH], bf16, name="e", tag="e")
                        nc.scalar.activation(out=e[:], in_=st[:],
                                             func=FT.Exp, scale=1.0 / 16.0)
                        # softmax denominator accumulates on the DVE
                        if mt == 0:
                            nc.vector.tensor_copy(esum[:], e[:])
                        else:
                            nc.vector.tensor_add(esum[:], esum[:], e[:])
                        pend.append((mt, e))
                        if len(pend) > 2:
                            pmt, pe_ = pend.pop(0)
                            zpv(pmt, pe_)
                    for fns in inject.values():
                        for fn in fns:
                            fn()
                    if tail is not None:
                        tail()
                    for pmt, pe_ in pend:
                        zpv(pmt, pe_)

                def make_epilogue(a, ch, pv, esum):
                    """Z partition-sum, reciprocal (Scalar ln+exp), 1/Z
                    broadcast, and the normalized X1 writes.  bf16 matmuls:
                    the esum cast error averages over 128 partitions
                    (~0.02%); the broadcast uses an exact hi+lo double-bf16
                    split.  Two stages so the PE work lands ~3.5us apart."""
                    esb = attnw.tile([128, CH], bf16, name="esb", tag="esb",
                                     bufs=1)
                    z = zbz.tile([1, CH], f32, name="z", tag="zbz")
                    lnz = attnw.tile([1, CH], f32, name="lnz", tag="lnz")
                    rz = attnw.tile([1, CH], f32, name="rz", tag="rz")
                    rzh = attnw.tile([1, CH], bf16, name="rzh", tag="rzh")
                    rzl = attnw.tile([1, CH], bf16, name="rzl", tag="rzl")
                    rbz = zbz.tile([128, CH], f32, name="rbz", tag="zbz")
                    rbzs = attnw.tile([128, CH], f32, name="rbzs",
                                      tag="rbzs", bufs=1)

                    def stage1():
                        nc.vector.tensor_copy(esb[:], esum[:])
                        nc.tensor.matmul(z[:], ones_b[:], esb[:],
                                         start=True, stop=True)
                        nc.scalar.activation(out=lnz[:], in_=z[:],
                                             func=FT.Ln)
                        nc.scalar.activation(out=rz[:], in_=lnz[:],
                                             func=FT.Exp, scale=-1.0)
                        nc.vector.tensor_copy(rzh[:], rz[:])
                        nc.vector.tensor_sub(rzl[:], rz[:], rzh[:])

                    def stage2():
                        nc.tensor.matmul(rbz[:], ones1b[:], rzh[:],
                                         start=True, stop=False)
                        nc.tensor.matmul(rbz[:], ones1b[:], rzl[:],
                                         start=False, stop=True)
                        # DVE reads at most one PSUM operand per op: stage
                        # the broadcast into SBUF before the pv multiplies
                        nc.vector.tensor_copy(rbzs[:], rbz[:])
                        if debug:
                            nc.sync.dma_start(out=dbg["Z"][a, ch],
                                              in_=rbzs[0:1, :])
                        for ct in range(2):
                            cit = 2 * a + ct
                            xv = X1[:, cit].rearrange("p (r c) -> p r c",
                                                      c=PADW)
                            nc.vector.tensor_mul(
                                xv[:, 1 + ch * 8:1 + ch * 8 + 8, 1:65],
                                pv[ct][:].rearrange("p (r w) -> p r w", w=64),
                                rbzs[:].rearrange("p (r w) -> p r w", w=64))
                    return stage1, stage2

                def pack_cc0(g):
                    # g=0: cits (2,3) [a=1]; g=1: cits (0,1) [a=0]
                    cits = (2, 3) if g == 0 else (0, 1)
                    st0 = stage.tile([128, 2, 2, 64], bf16, name="st0",
                                     tag="st0", bufs=1)
                    for t_, pr in ((0, 1), (1, 32)):
                        for ci, cit in enumerate(cits):
                            xv = X1[:, cit].rearrange("p (r c) -> p r c",
                                                      c=PADW)
                            nc.scalar.copy(st0[:, t_, ci],
                                           xv[:, pr, 1:65])
                    nc.gpsimd.dma_start(out=cc0i[g][:], in_=st0[:])
                    nc.gpsimd.collective_compute(
                        "AllGather", mybir.AluOpType.bypass,
                        ins=[cc0i[g][:]], outs=[cc0o[g][:]],
                        replica_groups=rpairs)

                def unpack_cc0(g):
                    # 2-slot select on the Pool engine
                    cits = (2, 3) if g == 0 else (0, 1)
                    G0 = ghw.tile([128, 2, 256], bf16, name="G0", tag="G0",
                                  bufs=1)
                    ap = cc0o[g][:]
                    nc.gpsimd.dma_start(out=G0[:], in_=bass.AP(
                        tensor=ap.tensor, offset=0,
                        ap=[[256, 128], [128 * 256, 2], [1, 256]]))
                    gap = G0[:]
                    for ci, cit in enumerate(cits):
                        for td, ts_ in ((0, 1), (1, 0)):
                            gsl = bass.AP(
                                tensor=gap.tensor,
                                offset=gap.offset + ts_ * 128 + ci * 64,
                                ap=[list(gap.ap[0]), [1, 64], [256, 2]])
                            prod = ghw.tile([128, 64, 2], bf16, name="prod0",
                                            tag="prod0")
                            nc.gpsimd.tensor_mul(prod[:], gsl, sel2[:, td])
                            nc.gpsimd.tensor_add(prod[:, :, 0:1],
                                                 prod[:, :, 0:1],
                                                 prod[:, :, 1:2])
                            xv = X1[:, cit].rearrange("p (r c) -> p r c",
                                                      c=PADW)
                            pr = 0 if td == 0 else 33
                            nc.gpsimd.tensor_copy(xv[:, pr, 1:65],
                                                  prod[:, :, 0])

                # chunk order: halo chunks (ch 0,3 of both a) first so the
                # halo collectives overlap mid-phase chunks; a=1 leads so
                # attention starts as soon as k1 (F1) + q2-ch0 (F2 piece 0)
                # are in; the remaining projections inject into the stream.
                chunk_seq = [(1, 0), (1, 3), (0, 0), (0, 3),
                             (1, 2), (1, 1), (0, 2), (0, 1)]

                inject_plan = {i: {} for i in range(8)}

                def add_inj(idx, mt, fn):
                    inject_plan[idx].setdefault(mt, []).append(fn)

                # idx0 (1,0): V1 per-mt (3 m-tiles ahead of its zpv use),
                # k1 ch2..7 paced with the F1 DMA, q2-ch3 for idx1
                for mt in range(MT):
                    add_inj(0, mt, (lambda m=mt: proj_v_mt(1, m)))
                for c_ in range(2, 8):
                    add_inj(0, 4 * (c_ - 1),
                            (lambda cc=c_: proj_qk_pack(kT[1], "k1", F1, cc)))
                add_inj(0, 27, lambda: proj_qk_pack(qT[1], "q2", F2, 3))
                # idx1 (1,3): k2 (for (0,*) scores), q1-ch0
                for c_ in range(8):
                    add_inj(1, 3 * c_,
                            (lambda cc=c_: proj_qk_pack(kT[0], "k2", F2, cc)))
                add_inj(1, 24, lambda: proj_qk_pack(qT[0], "q1", F1, 0))
                # idx2 (0,0): V2 per-mt, q1-ch3
                for mt in range(MT):
                    add_inj(2, mt, (lambda m=mt: proj_v_mt(0, m)))
                add_inj(2, 26, lambda: proj_qk_pack(qT[0], "q1", F1, 3))
                add_inj(3, 14, lambda: proj_qk_pack(qT[1], "q2", F2, 2))
                add_inj(4, 14, lambda: proj_qk_pack(qT[1], "q2", F2, 1))
                add_inj(5, 14, lambda: proj_qk_pack(qT[0], "q1", F1, 2))
                add_inj(6, 14, lambda: proj_qk_pack(qT[0], "q1", F1, 1))

                # pre-attention projections: k1 ch0/ch1, q2 ch0
                proj_qk_pack(kT[1], "k1", F1, 0)
                proj_qk_pack(kT[1], "k1", F1, 1)
                proj_qk_pack(qT[1], "q2", F2, 0)

                pending_epi = None   # (stage1, stage2) of previous chunk
                nlast = len(chunk_seq) - 1
                for idx, (a, ch) in enumerate(chunk_seq):
                    inject = {mt: list(fns)
                              for mt, fns in inject_plan[idx].items()}
                    if pending_epi is not None:
                        inject.setdefault(2, []).insert(0, pending_epi[0])
                        inject.setdefault(8, []).insert(0, pending_epi[1])
                    pv = [pvps.tile([128, CH], f32, name=f"pv{c_}",
                                    tag="pv") for c_ in range(2)]
                    esum = epool.tile([128, CH], f32, name="esum",
                                      tag="esum", bufs=2)
                    epi = make_epilogue(a, ch, pv, esum)
                    tail = epi[0] if idx == nlast else None
                    attn_chunk(a, ch, pv, esum, inject, tail)
                    if idx == nlast:
                        # stage2 right after the flush; conv1's first chunks
                        # (cits 2,3, which don't read these rows) overlap it
                        epi[1]()
                        pending_epi = None
                    else:
                        pending_epi = epi
                    if idx == 2:    # a=1 halo chunks written (idx 0,1)
                        pack_cc0(0)
                    if idx == 4:
                        unpack_cc0(0)
                        pack_cc0(1)   # a=0 halo chunks written (idx 2,3)
                    if idx == 6:
                        unpack_cc0(1)

            fmaps_ctx.__exit__(None, None, None)

            if debug:
                nc.sync.dma_start(out=dbg["qT1"][:], in_=qT[0][:])
                nc.sync.dma_start(out=dbg["kT2"][:], in_=kT[0][:])
                nc.sync.dma_start(out=dbg["V2"][:], in_=Vv[0][:])

            qkv_ctx.__exit__(None, None, None)

            if debug:
                nc.sync.dma_start(out=dbg["X1"][:], in_=X1[:])

            # =========== PHASE 3: conv stack ===========
            Xcur = X1
            with (
                tc.tile_pool(name="cpsum", bufs=8, space="PSUM") as cpsum,
                tc.tile_pool(name="convw", bufs=2) as convw,
            ):
                # read order: pss[3] first (needs only chunks 2,3 + bottom
                # ghost of the previous layer, the earliest-normalized rows)
                CH_ORDER = (3, 2, 1, 0)
                # close order for the last cit: halo chunks (0, 3) close
                # first so the pairwise halo AG fires half a cit early
                CLOSE_ORDER = (0, 3, 2, 1)

                def conv_layer_matmuls(li, cot, wct, cit_list, pss,
                                       start_cits, stop_cits, order,
                                       on_stop=None):
                    """Emit the 9-tap matmuls for the given cits of one cot.
                    start_cits/stop_cits: cit values that carry start/stop.
                    on_stop(ch) emits the psum eviction + local stats right
                    after chunk ch's accumulation group closes."""
                    parts = min(CONVS[li][1], 128)
                    for cit in cit_list:
                        xv = Xcur[:, cit].rearrange("p (r c) -> p r c",
                                                    c=PADW)
                        for ch in order:
                            for dy in range(3):
                                for dx in range(3):
                                    nc.tensor.matmul(
                                        pss[ch][:],
                                        wct[:, cit, dy, dx,
                                            cot * 128:cot * 128 + parts],
                                        xv[:, ch * 8 + dy:ch * 8 + dy + 8,
                                           dx:dx + 64],
                                        start=(cit in start_cits and dy == 0
                                               and dx == 0),
                                        stop=(cit in stop_cits and dy == 2
                                              and dx == 2))
                            if on_stop is not None and cit in stop_cits:
                                on_stop(ch)

                for li, (cin, cout, cit_n, cot_n) in enumerate(CONVS):
                    parts = min(cout, 128)
                    last = li == len(CONVS) - 1
                    wct = wc_sb[li]
                    bnt = bn_sb[li]  # [parts, 3(bc,g,bb), cot_n]
                    if not last:
                        Xnext = xpads.tile([128, cot_n, NPAD], bf16,
                                           name=f"X{li+2}", tag=f"X{li+2}")
                        # only the pad columns (0, 65) need zeroing: ghost
                        # rows are fully written by the halo path
                        for cot in range(cot_n):
                            xnf = Xnext[:, cot]
                            nc.vector.memset(bass.AP(
                                tensor=xnf.tensor, offset=xnf.offset,
                                ap=[list(xnf.ap[0]),
                                    [PADW, PADR], [65, 2]]), 0.0)
                    yf = [convw.tile([parts, NQ], f32, name=f"y{li}_{cot}",
                                     tag=f"y{li}_{cot}", bufs=1)
                          for cot in range(cot_n)]
                    pss_all = []
                    for cot in range(cot_n):
                        pss = {}
                        for ch in CH_ORDER:
                            pss[ch] = cpsum.tile([parts, CH], f32,
                                                 name=f"cps{cot}_{ch}",
                                                 tag="cps")
                        pss_all.append(pss)

                        # evictions (+conv bias) and local BN stats fire
                        # per chunk, as each accumulation group closes;
                        # halo AG fires once chunks 0 and 3 have closed
                        hstg = (stage.tile([parts, 2, 64], f32,
                                           name=f"hstg{li}_{cot}",
                                           tag="hstg")
                                if not last else None)
                        sstg = stage.tile([parts, 2], f32,
                                          name=f"sstg{li}_{cot}", tag="sstg")
                        bns = convw.tile([parts, 4, 6], f32, name="bns",
                                         tag="bns")
                        closed = []

                        def on_stop(ch, cot=cot, hstg=hstg, bns=bns,
                                    closed=closed):
                            nc.scalar.activation(
                                out=yf[cot][:, ch * CH:(ch + 1) * CH],
                                in_=pss[ch][:], func=FT.Identity,
                                bias=bnt[:, 0, cot:cot + 1])
                            # stats straight off the PSUM bank (no wait on
                            # the eviction); the conv bias only shifts the
                            # mean, corrected after the AllGather
                            nc.vector.bn_stats(
                                out=bns[:, ch], in_=pss[ch][:])
                            if not last:
                                if ch == 0:
                                    nc.vector.tensor_copy(
                                        hstg[:, 0], yf[cot][:, 0:64])
                                if ch == 3:
                                    nc.vector.tensor_copy(
                                        hstg[:, 1], yf[cot][:, NQ - 64:NQ])
                                closed.append(ch)
                                if set(closed) >= {0, 3} and \
                                        "fired" not in closed:
                                    closed.append("fired")
                                    nc.sync.dma_start(
                                        out=hli[li][cot][:],
                                        in_=hstg[:])
                                    nc.gpsimd.collective_compute(
                                        "AllGather", mybir.AluOpType.bypass,
                                        ins=[hli[li][cot][:]],
                                        outs=[hlo[li][cot][:]],
                                        replica_groups=rpairs)

                        if li == 1:
                            # conv2: consume conv1-cot0 channels (cit 0) for
                            # all pss chunks first; conv1-cot1's stats
                            # AllReduce hides behind them.
                            conv_layer_matmuls(li, cot, wct, [0], pss,
                                               {0}, set(), CH_ORDER)
                            conv_layer_matmuls(li, cot, wct, [1], pss,
                                               set(), {1}, CLOSE_ORDER,
                                               on_stop=on_stop)
                        elif li == 0:
                            # conv1: a=1 channels (X1 cits 2,3 -- done early
                            # in the attention chunk order) first
                            conv_layer_matmuls(li, cot, wct, [2, 3, 0], pss,
                                               {2}, set(), CH_ORDER)
                            conv_layer_matmuls(li, cot, wct, [1], pss,
                                               set(), {1}, CLOSE_ORDER,
                                               on_stop=on_stop)
                        else:
                            conv_layer_matmuls(li, cot, wct, [0], pss,
                                               {0}, {0}, CH_ORDER,
                                               on_stop=on_stop)

                        # send per-core (mean, var + mean^2) unscaled;
                        # equal counts per rank, so the receive just
                        # averages over the 8 slots
                        nc.vector.bn_aggr(out=sstg[:], in_=bns[:])
                        m2 = convw.tile([parts, 1], f32, name="m2", tag="m2")
                        nc.vector.tensor_mul(m2[:], sstg[:, 0:1],
                                             sstg[:, 0:1])
                        nc.vector.tensor_add(sstg[:, 1:2], sstg[:, 1:2],
                                             m2[:])
                        nc.sync.dma_start(out=sli[li][cot][:], in_=sstg[:])
                        # AllGather + receive-side reduce: the 8-rank
                        # AG mesh measures ~4us faster than AllReduce
                        nc.gpsimd.collective_compute(
                            "AllGather", mybir.AluOpType.bypass,
                            ins=[sli[li][cot][:]], outs=[slo[li][cot][:]],
                            replica_groups=replica8)

                        # ---- receive path, emitted per cot so cot0's
                        # normalize overlaps cot1's matmul stream (keeps
                        # sync's DMA queue free of priority inversions:
                        # halo0, stats0, sGL0, halo1, stats1, sGL1) ----
                        if not last:
                            hGL = convw.tile([parts, 2, 128], f32,
                                             name=f"hGL{li}_{cot}",
                                             tag="hGL")
                            hap = hlo[li][cot][:]
                            nc.gpsimd.dma_start(out=hGL[:], in_=bass.AP(
                                tensor=hap.tensor, offset=0,
                                ap=[[128, parts], [parts * 128, 2],
                                    [1, 128]]))
                            hgap = hGL[:]
                        sGL = convw.tile([parts, 8, 2], f32, name="sGL",
                                         tag="sGL")
                        sap = slo[li][cot][:]
                        nc.sync.dma_start(out=sGL[:], in_=bass.AP(
                            tensor=sap.tensor, offset=0,
                            ap=[[2, parts], [parts * 2, 8], [1, 2]]))
                        sgap = sGL[:]
                        ssl = bass.AP(tensor=sgap.tensor, offset=sgap.offset,
                                      ap=[list(sgap.ap[0]), [1, 2], [2, 8]])
                        tot = convw.tile([parts, 2], f32, name="tot",
                                         tag="tot")
                        nc.vector.reduce_sum(tot[:], ssl,
                                             axis=mybir.AxisListType.X)
                        # scale/bias: rstd = (Q/N - (S/N)^2 + eps)^-1/2
                        ms = convw.tile([parts, 2], f32, name="ms", tag="ms")
                        nc.vector.tensor_scalar_mul(ms[:], tot[:],
                                                    1.0 / 8.0)
                        m2b = convw.tile([parts, 1], f32, name="m2b",
                                         tag="m2b")
                        nc.vector.tensor_mul(m2b[:], ms[:, 0:1], ms[:, 0:1])
                        var = convw.tile([parts, 1], f32, name="var",
                                         tag="var")
                        nc.vector.tensor_sub(var[:], ms[:, 1:2], m2b[:])
                        lnv = convw.tile([parts, 1], f32, name="lnv",
                                         tag="lnv")
                        nc.scalar.activation(out=lnv[:], in_=var[:],
                                             func=FT.Ln, bias=epsb[:parts])
                        rstd = convw.tile([parts, 1], f32, name="rstd",
                                          tag="rstd")
                        nc.scalar.activation(out=rstd[:], in_=lnv[:],
                                             func=FT.Exp, scale=-0.5)
                        scl = convw.tile([parts, 1], f32, name="scl",
                                         tag="scl")
                        nc.vector.tensor_mul(scl[:], bnt[:, 1, cot:cot + 1],
                                             rstd[:])
                        bia = convw.tile([parts, 1], f32, name="bia",
                                         tag="bia")
                        meanY = convw.tile([parts, 1], f32, name="meanY",
                                           tag="meanY")
                        nc.vector.tensor_add(meanY[:], ms[:, 0:1],
                                             bnt[:, 0, cot:cot + 1])
                        nc.vector.tensor_mul(bia[:], meanY[:], scl[:])
                        nc.vector.tensor_sub(bia[:], bnt[:, 2, cot:cot + 1],
                                             bia[:])
                        if debug and li == 0:
                            nc.sync.dma_start(out=dbg["SC1"][cot, :, 0:1],
                                              in_=scl[:])
                            nc.sync.dma_start(out=dbg["SC1"][cot, :, 1:2],
                                              in_=bia[:])

                        if last:
                            # final relu per chunk + output DMA on 4 queues
                            dma_engines = [nc.sync, nc.gpsimd]
                            for i, ch in enumerate((0, 1)):
                                outf = convw.tile([parts, 2 * CH], f32,
                                                  name=f"outf{ch}",
                                                  tag="outf", bufs=2)
                                nc.scalar.activation(
                                    out=outf[:],
                                    in_=yf[cot][:, 2 * ch * CH:
                                                (2 * ch + 2) * CH],
                                    func=FT.Relu, scale=scl[:], bias=bia[:])
                                dma_engines[i].dma_start(
                                    out=d_out[:, 2 * ch * CH:
                                              (2 * ch + 2) * CH],
                                    in_=outf[:])
                        else:
                            xv = Xnext[:, cot].rearrange("p (r c) -> p r c",
                                                         c=PADW)

                            def norm_chunk(ch):
                                nc.scalar.activation(
                                    out=xv[:parts, 1 + ch * 8:9 + ch * 8,
                                           1:65],
                                    in_=yf[cot][:, ch * CH:(ch + 1) * CH]
                                    .rearrange("p (r w) -> p r w", w=64),
                                    func=FT.Relu, scale=scl[:], bias=bia[:])

                            def ghost_row(td, ts_):
                                # ghost rows: 2-slot select from the
                                # pairwise-gathered buffer
                                gsl = bass.AP(
                                    tensor=hgap.tensor,
                                    offset=hgap.offset + ts_ * 64,
                                    ap=[list(hgap.ap[0]), [1, 64],
                                        [128, 2]])
                                prod = convw.tile([parts, 64, 2], f32,
                                                  name="prodL", tag="prodL")
                                nc.vector.tensor_mul(prod[:], gsl,
                                                     selF2[:parts, td])
                                nc.vector.tensor_add(prod[:, :, 0:1],
                                                     prod[:, :, 0:1],
                                                     prod[:, :, 1:2])
                                gb = convw.tile([parts, 64], bf16,
                                                name="gbL", tag="gbL")
                                nc.scalar.activation(out=gb[:],
                                                     in_=prod[:, :, 0],
                                                     func=FT.Relu,
                                                     scale=scl[:], bias=bia[:])
                                pr = 0 if td == 0 else 33
                                nc.vector.tensor_mul(xv[:parts, pr, 1:65],
                                                     gb[:],
                                                     rowm[:parts, td])

                            # order matched to the next layer's CH_ORDER
                            # (3,2,1,0): pss[3] needs chunks 2,3 + bottom
                            # ghost; pss[2] adds chunk 1; pss[0] is last.
                            norm_chunk(3)
                            norm_chunk(2)
                            ghost_row(1, 0)   # bottom ghost (row 33)
                            norm_chunk(1)
                            norm_chunk(0)
                            ghost_row(0, 1)   # top ghost (row 0)
                    if debug and li == 0:
                        for cot in range(cot_n):
                            nc.sync.dma_start(out=dbg["Y1"][cot],
                                              in_=yf[cot][:])
                        if not last:
                            nc.sync.dma_start(out=dbg["X2"][:], in_=Xnext[:])
                    if not last:
                        Xcur = Xnext

    n = _split_excess_waits(nc, 1)
    return nc, n


def _shard_inputs(inputs):
    """Build the 8 per-core input maps from the full problem inputs."""
    bf = ml_dtypes.bfloat16
    fm1 = np.asarray(inputs["feature_map1"], np.float32)
    fm2 = np.asarray(inputs["feature_map2"], np.float32)

    def pshuf(a2d):  # [2*128, X] -> [128, 2*X] partition-major
        n2, x = a2d.shape
        kt = n2 // 128
        return np.ascontiguousarray(
            a2d.reshape(kt, 128, x).transpose(1, 0, 2).reshape(128, kt * x))

    shared = {}
    for nm in ("q1", "k2", "v2", "q2", "k1", "v1"):
        wT = np.asarray(inputs[f"{nm}_w"], np.float32).T  # [in, out]
        shared[f"w_{nm}"] = pshuf(wT).astype(bf)
    for nm in ("q1", "k2", "q2", "k1"):
        b = np.asarray(inputs[f"{nm}_b"], np.float32)
        shared[f"b_{nm}"] = np.ascontiguousarray(b.reshape(2, 128).T)
    shared["bv1r"] = np.tile(np.asarray(inputs["v1_b"], np.float32)[None, :],
                             (128, 1))
    shared["bv2r"] = np.tile(np.asarray(inputs["v2_b"], np.float32)[None, :],
                             (128, 1))
    for li, (cin, cout, cit_n, cot_n) in enumerate(CONVS):
        wc = np.asarray(inputs[f"conv{li+1}_w"], np.float32)  # [co, ci, 3, 3]
        # -> [p, cit, ky, kx, co]
        arr = wc.transpose(1, 2, 3, 0).reshape(cit_n, 128, 3, 3, cout)
        arr = arr.transpose(1, 0, 2, 3, 4).reshape(128, -1)
        shared[f"wc{li}"] = np.ascontiguousarray(arr).astype(bf)
        parts = min(cout, 128)
        cot_nn = cout // parts
        trio = np.stack([
            np.asarray(inputs[f"conv{li+1}_b"], np.float32),
            np.asarray(inputs[f"bn{li+1}_g"], np.float32),
            np.asarray(inputs[f"bn{li+1}_b"], np.float32),
        ])  # [3, cout]
        # -> [parts, 3, cot_n] -> [parts, 3*cot_n]
        arr = trio.reshape(3, cot_nn, parts).transpose(2, 0, 1)
        shared[f"bn{li}"] = np.ascontiguousarray(arr.reshape(parts, -1))

    in_maps = []
    for r in range(N_CORES):
        b, half = divmod(r, 2)
        h0 = 32 * half
        m = dict(shared)
        # roll rows so this core's query rows are columns 0:2048
        m["f1"] = pshuf(np.roll(fm1[b], -h0, axis=1).reshape(C, HW)).astype(bf)
        m["f2"] = pshuf(np.roll(fm2[b], -h0, axis=1).reshape(C, HW)).astype(bf)
        # ghost row selection over the 2 pair slots:
        # dest td=0 (top ghost) / td=1 (bottom ghost)
        sel = np.zeros((2, 2), np.float32)
        pslot = 1 - (r & 1)     # partner's slot within the pair
        if half == 0:
            sel[1, pslot] = 1.0   # bottom ghost <- partner's top row
        else:
            sel[0, pslot] = 1.0   # top ghost <- partner's bottom row
        selfull = np.broadcast_to(sel[None, :, None, :],
                                  (128, 2, 64, 2)).copy()
        m["sel2"] = selfull.astype(bf)
        m["selF2"] = selfull.astype(np.float32)
        rowmask = sel.sum(-1)  # [2]
        m["rowm"] = np.broadcast_to(rowmask[None, :, None],
                                    (128, 2, 64)).copy().astype(bf)
        in_maps.append(m)
    return in_maps


def _get_program(debug=False):
    key = ("dbg" if debug else "rel")
    if key not in _CACHE:
        _CACHE[key] = _build_program(debug=debug)
    return _CACHE[key]


def run(inputs, trace=False, debug=False):
    from concourse.bass_utils import run_bass_kernel_spmd
    nc, _ = _get_program(debug=debug)
    in_maps = _shard_inputs(inputs)
    res = run_bass_kernel_spmd(nc, in_maps, list(range(N_CORES)), trace=trace)
    out = np.zeros((B, 64, H, W), np.float32)
    for r in range(N_CORES):
        b, half = divmod(r, 2)
        h0 = 32 * half
        out[b, :, h0:h0 + 32, :] = res.results[r]["yout"].reshape(64, 32, 64)
    return out, res


def kernel(**inputs):
    out, _ = run(inputs, trace=False)
    return out


# revision 29
# speedup vs baseline: 1.0336x; 1.0336x over previous
"""Trainium2 Bass kernel for nn_CrossAttentionFusion.

Sharding: 8 cores = (batch b in 0..4) x (image-row half in 0..2).
Each core computes cross-attention for its 2048 query pixels (K/V over the
full 4096 pixels of its batch element, K/V compute replicated within the
pair), then the conv+BN+relu stack for its half of the image.

V2 structure:
  - projections are fused into the attention stream (no separate PSUM
    phase): k1/q2 projections start as soon as their first feature-map
    DMA pieces land (~16us), attention chunk (1,0) follows immediately,
    and the remaining projections (q1, k2, V1, V2, rest of q2) are
    injected between attention m-tiles so the PE never drains.
  - DMAs are issued in consumption order (w_k1/w_q2/w_v1 first, then
    interleaved F1/F2 column pieces, conv weights last).
  - softmax reciprocal on Scalar (ln+exp) instead of the DVE (3.3us).
  - halo rows (attention X1 ghosts + conv ghosts) exchanged with
    PAIRWISE AllGathers; BN statistics with 8-rank AllReduce of
    (sum, sumsq) so no gather/reduce is needed on the receive side.
  - conv halo AG fires when chunks 0/3 of the last cit close (close
    order 0,3,2,1), half a cit before the stats AllReduce.
  - conv1 consumes the a=1 channels (X1 cits 2,3 -- finished early in
    the chunk order) first, so its first psum groups can start while
    the last attention chunks are still streaming.
"""

import os
import numpy as np
import ml_dtypes

N_CORES = 8
B, C, H, W = 4, 256, 64, 64
HW = H * W              # 4096
NQ = 2048               # queries per core
CH = 512                # free-dim chunk (1 PSUM bank of f32)
NCH = NQ // CH          # 4
MT = HW // 128          # 32 m-tiles
EPS = 1e-5
PADW = 66
PADR = 34
NPAD = PADR * PADW      # 2244

# conv layer specs: (Cin, Cout, cin_tiles, cout_tiles)
CONVS = [(512, 256, 4, 2), (256, 128, 2, 1), (128, 64, 1, 1)]

_CACHE = {}


# --------------------------------------------------------------------------
# walrus in this container rejects >1 sync wait per instruction; split extras
# onto same-engine NOPs placed immediately before the offending instruction.
def _split_excess_waits(nc, max_waits=1):
    import bass_rust
    eng_map = dict(nc.engines)
    n_split = 0
    for fn in nc.m.functions:
        for bb in fn.blocks:
            lst = bb.instructions
            i = 0
            while i < len(lst):
                inst = lst[i]
                si = inst.sync_info
                if si is not None and len(si.on_wait) > max_waits:
                    waits = list(si.on_wait)
                    keep = waits[-max_waits:]
                    extra = waits[:-max_waits]
                    inst.sync_info = bass_rust.SyncInfo(
                        on_wait=keep, on_update=list(si.on_update))
                    eng = eng_map[inst.engine]
                    pos = i
                    for j in range(0, len(extra), max_waits):
                        chunk = extra[j:j + max_waits]
                        bi = eng.nop()
                        nop_inst = bi.ins
                        cur = nc.cur_bb.bb.instructions
                        assert cur[-1] is nop_inst
                        cur.pop()
                        nop_inst.sync_info = bass_rust.SyncInfo(
                            on_wait=chunk, on_update=[])
                        lst.insert(pos, nop_inst)
                        pos += 1
                        i += 1
                    n_split += 1
                i += 1
    return n_split


def _build_program(debug=False):
    import concourse.bass as bass
    import concourse.mybir as mybir
    import concourse.tile as tile

    f32 = mybir.dt.float32
    bf16 = mybir.dt.bfloat16
    FT = mybir.ActivationFunctionType

    nc = bass.Bass()

    # ---------------- DRAM I/O (all host pre-shuffled, contiguous) --------
    d_f1 = nc.dram_tensor("f1", [128, 2 * HW], bf16, kind="ExternalInput")
    d_f2 = nc.dram_tensor("f2", [128, 2 * HW], bf16, kind="ExternalInput")
    d_w = {}
    d_b = {}
    for nm in ("q1", "k2", "v2", "q2", "k1", "v1"):
        d_w[nm] = nc.dram_tensor(f"w_{nm}", [128, 2 * C], bf16,
                                 kind="ExternalInput")
    for nm in ("q1", "k2", "q2", "k1"):
        d_b[nm] = nc.dram_tensor(f"b_{nm}", [128, 2], f32,
                                 kind="ExternalInput")
    d_bv1r = nc.dram_tensor("bv1r", [128, C], f32, kind="ExternalInput")
    d_bv2r = nc.dram_tensor("bv2r", [128, C], f32, kind="ExternalInput")

    d_wc, d_bn = [], []
    for li, (cin, cout, cit_n, cot_n) in enumerate(CONVS):
        d_wc.append(nc.dram_tensor(f"wc{li}", [128, cit_n * 9 * cout], bf16,
                                   kind="ExternalInput"))
        parts = min(cout, 128)
        # bc, gamma, beta stacked: [parts, 3*cot_n]
        d_bn.append(nc.dram_tensor(f"bn{li}", [parts, 3 * cot_n], f32,
                                   kind="ExternalInput"))

    d_sel2 = nc.dram_tensor("sel2", [128, 2, 64, 2], bf16, kind="ExternalInput")
    d_self = nc.dram_tensor("selF2", [128, 2, 64, 2], f32, kind="ExternalInput")
    d_rowm = nc.dram_tensor("rowm", [128, 2, 64], bf16, kind="ExternalInput")

    d_out = nc.dram_tensor("yout", [64, 32 * 64], f32, kind="ExternalOutput")

    # collective buffers
    replica8 = [list(range(N_CORES))]
    rpairs = [[2 * i, 2 * i + 1] for i in range(N_CORES // 2)]
    # attention halo rows: pairwise AG, one per a-half (2 cits x 2 rows x 64)
    cc0i = [nc.dram_tensor(f"cc0i{g}", [128, 256], bf16) for g in range(2)]
    cc0o = [nc.dram_tensor(f"cc0o{g}", [2, 128, 256], bf16) for g in range(2)]
    # conv BN stats: 8-rank AllReduce of (sum, sumsq)
    sli, slo = [], []
    # conv halo rows: pairwise AG of (row1, row32) f32
    hli, hlo = [], []
    for li, (cin, cout, cit_n, cot_n) in enumerate(CONVS):
        parts = min(cout, 128)
        sli.append([nc.dram_tensor(f"s{li}i{c_}", [parts, 2], f32)
                    for c_ in range(cot_n)])
        slo.append([nc.dram_tensor(f"s{li}o{c_}", [8, parts, 2], f32,
                                   addr_space="Shared")
                    for c_ in range(cot_n)])
        if li < 2:
            hli.append([nc.dram_tensor(f"h{li}i{c_}", [parts, 128], f32)
                        for c_ in range(cot_n)])
            hlo.append([nc.dram_tensor(f"h{li}o{c_}", [2, parts, 128], f32)
                        for c_ in range(cot_n)])

    dbg = {}
    if debug:
        dbg["qT1"] = nc.dram_tensor("d_qT1", [128, 2, NQ], bf16, kind="ExternalOutput")
        dbg["kT2"] = nc.dram_tensor("d_kT2", [128, 2, HW], bf16, kind="ExternalOutput")
        dbg["V2"] = nc.dram_tensor("d_V2", [128, MT, C], bf16, kind="ExternalOutput")
        dbg["Z"] = nc.dram_tensor("d_Z", [2, NCH, CH], f32, kind="ExternalOutput")
        dbg["X1"] = nc.dram_tensor("d_X1", [128, 4, NPAD], bf16, kind="ExternalOutput")
        dbg["Y1"] = nc.dram_tensor("d_Y1", [2, 128, NQ], f32, kind="ExternalOutput")
        dbg["SC1"] = nc.dram_tensor("d_SC1", [2, 128, 2], f32, kind="ExternalOutput")
        dbg["X2"] = nc.dram_tensor("d_X2", [128, 2, NPAD], bf16, kind="ExternalOutput")

    with tile.TileContext(nc) as tc:
        with (
            tc.tile_pool(name="consts", bufs=1) as consts,
            tc.tile_pool(name="xpads", bufs=1) as xpads,
            tc.tile_pool(name="stage", bufs=2) as stage,
        ):
            # qkv pool outlives fmaps (LIFO pool release)
            qkv_ctx = tc.tile_pool(name="qkv", bufs=1)
            qkv = qkv_ctx.__enter__()
            qT = [qkv.tile([128, 2, NQ], bf16, name="qT1", tag="qT1"),
                  qkv.tile([128, 2, NQ], bf16, name="qT2", tag="qT2")]
            kT = [qkv.tile([128, 2, HW], bf16, name="kT2", tag="kT2"),
                  qkv.tile([128, 2, HW], bf16, name="kT1", tag="kT1")]
            Vv = [qkv.tile([128, MT, C], bf16, name="V2", tag="V2"),
                  qkv.tile([128, MT, C], bf16, name="V1", tag="V1")]

            fmaps_ctx = tc.tile_pool(name="fmaps", bufs=1)
            fmaps = fmaps_ctx.__enter__()
            F1 = fmaps.tile([128, 2, HW], bf16, name="F1")
            F2 = fmaps.tile([128, 2, HW], bf16, name="F2")

            # ---- const tiles (allocate first, DMA in priority order) ----
            w_sb = {}
            for nm in ("q1", "k2", "v2", "q2", "k1", "v1"):
                w_sb[nm] = consts.tile([128, 2, C], bf16, name=f"w_{nm}",
                                       tag=f"w_{nm}")
            b_sb = {}
            for nm in ("q1", "k2", "q2", "k1"):
                b_sb[nm] = consts.tile([128, 2], f32, name=f"b_{nm}",
                                       tag=f"b_{nm}")
            bv1r = consts.tile([128, C], f32, name="bv1r")
            bv2r = consts.tile([128, C], f32, name="bv2r")
            sel2 = consts.tile([128, 2, 64, 2], bf16, name="sel2")
            selF2 = consts.tile([128, 2, 64, 2], f32, name="selF2")
            rowm = consts.tile([128, 2, 64], bf16, name="rowm")
            wc_sb = []
            bn_sb = []
            for li, (cin, cout, cit_n, cot_n) in enumerate(CONVS):
                parts = min(cout, 128)
                wc_sb.append(consts.tile([128, cit_n, 3, 3, cout], bf16,
                                         name=f"wc{li}", tag=f"wc{li}"))
                bn_sb.append(consts.tile([parts, 3, cot_n], f32,
                                         name=f"bn{li}", tag=f"bn{li}"))

            # ---- DMA kicks in consumption order ------------------------
            # NEVER kick from Scalar/Vector/Tensor: a SWDGE dma_start costs
            # the issuing engine ~0.7us and the projection evictions / exp
            # stream sit behind it in program order (measured: Scalar was
            # blocked until 35us when it carried kicks).
            dmae = [nc.sync, nc.gpsimd]
            _ki = [0]

            def kick(out, in_):
                dmae[_ki[0] % 2].dma_start(out=out, in_=in_)
                _ki[0] += 1

            PC = 1024       # fmap DMA piece: [128, 1, 1024] = 2KB/partition

            def fpiece(Ft, dt, c, kt):
                kick(Ft[:, kt, c * PC:(c + 1) * PC],
                     dt[:, kt * HW + c * PC:kt * HW + (c + 1) * PC])

            # first-needed: k1 (F1 c0), q2 ch0 (F2 c0), v1 (F1)
            kick(w_sb["k1"][:], d_w["k1"][:])
            kick(b_sb["k1"][:], d_b["k1"][:])
            fpiece(F1, d_f1, 0, 0)
            fpiece(F1, d_f1, 0, 1)
            kick(w_sb["q2"][:], d_w["q2"][:])
            kick(b_sb["q2"][:], d_b["q2"][:])
            fpiece(F2, d_f2, 0, 0)
            fpiece(F2, d_f2, 0, 1)
            kick(w_sb["v1"][:], d_w["v1"][:])
            kick(bv1r[:], d_bv1r[:])
            for c in range(1, 4):
                fpiece(F1, d_f1, c, 0)
                fpiece(F1, d_f1, c, 1)
            kick(w_sb["q1"][:], d_w["q1"][:])
            kick(b_sb["q1"][:], d_b["q1"][:])
            kick(w_sb["k2"][:], d_w["k2"][:])
            kick(b_sb["k2"][:], d_b["k2"][:])
            kick(w_sb["v2"][:], d_w["v2"][:])
            kick(bv2r[:], d_bv2r[:])
            for c in range(1, 4):
                fpiece(F2, d_f2, c, 0)
                fpiece(F2, d_f2, c, 1)
            kick(sel2[:], d_sel2[:])
            kick(selF2[:], d_self[:])
            kick(rowm[:], d_rowm[:])
            for li in range(len(CONVS)):
                kick(bn_sb[li][:], d_bn[li][:])
            for li in range(len(CONVS)):
                kick(wc_sb[li][:], d_wc[li][:])

            # ---- constants ----
            ones_b = consts.tile([128, 1], bf16, name="ones_b")
            nc.vector.memset(ones_b, 1.0)
            ones1b = consts.tile([1, 128], bf16, name="ones1b")
            nc.vector.memset(ones1b, 1.0)
            epsb = consts.tile([128, 1], f32, name="epsb")
            nc.vector.memset(epsb, EPS)

            # ---- persistent activations ----
            X1 = xpads.tile([128, 4, NPAD], bf16, name="X1")
            nc.vector.memset(X1, 0.0)

            # =========== fused projections + attention ===========
            with (
                tc.tile_pool(name="stps", bufs=3, space="PSUM") as stps,
                tc.tile_pool(name="pvps", bufs=4, space="PSUM") as pvps,
                tc.tile_pool(name="zbz", bufs=1, space="PSUM") as zbz,
                tc.tile_pool(name="attnw", bufs=2) as attnw,
                tc.tile_pool(name="epool", bufs=4) as epool,
                tc.tile_pool(name="ghw", bufs=2) as ghw,
            ):
                def proj_qk(dst, wname, Fsrc, ct, ch):
                    ps = stps.tile([128, CH], f32, name="pps", tag="st")
                    wt = w_sb[wname]
                    for kt in range(2):
                        nc.tensor.matmul(
                            ps[:], wt[:, kt, ct * 128:(ct + 1) * 128],
                            Fsrc[:, kt, ch * CH:(ch + 1) * CH],
                            start=(kt == 0), stop=(kt == 1))
                    nc.scalar.activation(
                        out=dst[:, ct, ch * CH:(ch + 1) * CH], in_=ps[:],
                        func=FT.Identity, bias=b_sb[wname][:, ct:ct + 1])

                def proj_qk_pack(dst, wname, Fsrc, ch):
                    for ct in range(2):
                        proj_qk(dst, wname, Fsrc, ct, ch)

                def proj_v_mt(a, mt):
                    dst, wname, Fsrc, bvr = (
                        (Vv[1], "v1", F1, bv1r) if a == 1
                        else (Vv[0], "v2", F2, bv2r))
                    ps = stps.tile([128, CH], f32, name="pps", tag="st")
                    wt = w_sb[wname]
                    for kt in range(2):
                        nc.tensor.matmul(
                            ps[:, 0:C], Fsrc[:, kt, mt * 128:(mt + 1) * 128],
                            wt[:, kt, :], start=(kt == 0), stop=(kt == 1))
                    nc.vector.tensor_add(dst[:, mt, :], ps[:, 0:C], bvr[:])

                def attn_chunk(a, ch, pv, esum, inject=None, tail=None):
                    """Emit one attention chunk; inject[mt] is a list of
                    hooks emitted before m-tile mt's score matmuls (used to
                    software-pipeline the previous chunk's epilogue and the
                    remaining projections into this chunk's score stream).
                    tail() (if given) is emitted between the mt loop and the
                    final zpv flush, so the flush matmuls cover its latency."""
                    inject = dict(inject or {})
                    Q, K, V = qT[a], kT[a], Vv[a]

                    def zpv(pmt, e):
                        for ct in range(2):
                            nc.tensor.matmul(
                                pv[ct][:],
                                V[:, pmt, ct * 128:(ct + 1) * 128], e[:],
                                start=(pmt == 0), stop=(pmt == MT - 1))

                    pend = []
                    for mt in range(MT):
                        for fn in inject.pop(mt, ()):
                            fn()
                        st = stps.tile([128, CH], f32, name="st", tag="st")
                        for kt in range(2):
                            nc.tensor.matmul(
                                st[:], K[:, kt, mt * 128:(mt + 1) * 128],
                                Q[:, kt, ch * CH:(ch + 1) * CH],
                                start=(kt == 0), stop=(kt == 1))
                        e = epool.tile([128, CH], bf16, name="e", tag="e")
                        nc.scalar.activation(out=e[:], in_=st[:],
                                             func=FT.Exp, scale=1.0 / 16.0)
                        # softmax denominator accumulates on the DVE
                        if mt == 0:
                            nc.vector.tensor_copy(esum[:], e[:])
                        else:
                            nc.vector.tensor_add(esum[:], esum[:], e[:])
                        pend.append((mt, e))
                        if len(pend) > 2:
                            pmt, pe_ = pend.pop(0)
                            zpv(pmt, pe_)
                    for fns in inject.values():
                        for fn in fns:
                            fn()
                    if tail is not None:
                        tail()
                    for pmt, pe_ in pend:
                        zpv(pmt, pe_)

                def make_epilogue(a, ch, pv, esum):
                    """Z partition-sum, reciprocal (Scalar ln+exp), 1/Z
                    broadcast, and the normalized X1 writes.  bf16 matmuls:
                    the esum cast error averages over 128 partitions
                    (~0.02%); the broadcast uses an exact hi+lo double-bf16
                    split.  Two stages so the PE work lands ~3.5us apart."""
                    esb = attnw.tile([128, CH], bf16, name="esb", tag="esb",
                                     bufs=1)
                    z = zbz.tile([1, CH], f32, name="z", tag="zbz")
                    lnz = attnw.tile([1, CH], f32, name="lnz", tag="lnz")
                    rz = attnw.tile([1, CH], f32, name="rz", tag="rz")
                    rzh = attnw.tile([1, CH], bf16, name="rzh", tag="rzh")
                    rzl = attnw.tile([1, CH], bf16, name="rzl", tag="rzl")
                    rbz = zbz.tile([128, CH], f32, name="rbz", tag="zbz")
                    rbzs = attnw.tile([128, CH], f32, name="rbzs",
                                      tag="rbzs", bufs=1)

                    def stage1():
                        nc.vector.tensor_copy(esb[:], esum[:])
                        nc.tensor.matmul(z[:], ones_b[:], esb[:],
                                         start=True, stop=True)
                        nc.scalar.activation(out=lnz[:], in_=z[:],
                                             func=FT.Ln)
                        nc.scalar.activation(out=rz[:], in_=lnz[:],
                                             func=FT.Exp, scale=-1.0)
                        nc.vector.tensor_copy(rzh[:], rz[:])
                        nc.vector.tensor_sub(rzl[:], rz[:], rzh[:])

                    def stage2():
                        nc.tensor.matmul(rbz[:], ones1b[:], rzh[:],
                                         start=True, stop=False)
                        nc.tensor.matmul(rbz[:], ones1b[:], rzl[:],
                                         start=False, stop=True)
                        # DVE reads at most one PSUM operand per op: stage
                        # the broadcast into SBUF before the pv multiplies
                        nc.vector.tensor_copy(rbzs[:], rbz[:])
                        if debug:
                            nc.sync.dma_start(out=dbg["Z"][a, ch],
                                              in_=rbzs[0:1, :])
                        for ct in range(2):
                            cit = 2 * a + ct
                            xv = X1[:, cit].rearrange("p (r c) -> p r c",
                                                      c=PADW)
                            nc.vector.tensor_mul(
                                xv[:, 1 + ch * 8:1 + ch * 8 + 8, 1:65],
                                pv[ct][:].rearrange("p (r w) -> p r w", w=64),
                                rbzs[:].rearrange("p (r w) -> p r w", w=64))
                    return stage1, stage2

                def pack_cc0(g):
                    # g=0: cits (2,3) [a=1]; g=1: cits (0,1) [a=0]
                    cits = (2, 3) if g == 0 else (0, 1)
                    st0 = stage.tile([128, 2, 2, 64], bf16, name="st0",
                                     tag="st0", bufs=1)
                    for t_, pr in ((0, 1), (1, 32)):
                        for ci, cit in enumerate(cits):
                            xv = X1[:, cit].rearrange("p (r c) -> p r c",
                                                      c=PADW)
                            nc.scalar.copy(st0[:, t_, ci],
                                           xv[:, pr, 1:65])
                    nc.gpsimd.dma_start(out=cc0i[g][:], in_=st0[:])
                    nc.gpsimd.collective_compute(
                        "AllGather", mybir.AluOpType.bypass,
                        ins=[cc0i[g][:]], outs=[cc0o[g][:]],
                        replica_groups=rpairs)

                def unpack_cc0(g):
                    # 2-slot select on the Pool engine
                    cits = (2, 3) if g == 0 else (0, 1)
                    G0 = ghw.tile([128, 2, 256], bf16, name="G0", tag="G0",
                                  bufs=1)
                    ap = cc0o[g][:]
                    nc.gpsimd.dma_start(out=G0[:], in_=bass.AP(
                        tensor=ap.tensor, offset=0,
                        ap=[[256, 128], [128 * 256, 2], [1, 256]]))
                    gap = G0[:]
                    for ci, cit in enumerate(cits):
                        for td, ts_ in ((0, 1), (1, 0)):
                            gsl = bass.AP(
                                tensor=gap.tensor,
                                offset=gap.offset + ts_ * 128 + ci * 64,
                                ap=[list(gap.ap[0]), [1, 64], [256, 2]])
                            prod = ghw.tile([128, 64, 2], bf16, name="prod0",
                                            tag="prod0")
                            nc.gpsimd.tensor_mul(prod[:], gsl, sel2[:, td])
                            nc.gpsimd.tensor_add(prod[:, :, 0:1],
                                                 prod[:, :, 0:1],
                                                 prod[:, :, 1:2])
                            xv = X1[:, cit].rearrange("p (r c) -> p r c",
                                                      c=PADW)
                            pr = 0 if td == 0 else 33
                            nc.gpsimd.tensor_copy(xv[:, pr, 1:65],
                                                  prod[:, :, 0])

                # chunk order: halo chunks (ch 0,3 of both a) first so the
                # halo collectives overlap mid-phase chunks; a=1 leads so
                # attention starts as soon as k1 (F1) + q2-ch0 (F2 piece 0)
                # are in; the remaining projections inject into the stream.
                chunk_seq = [(1, 0), (1, 3), (0, 0), (0, 3),
                             (1, 2), (1, 1), (0, 2), (0, 1)]

                inject_plan = {i: {} for i in range(8)}

                def add_inj(idx, mt, fn):
                    inject_plan[idx].setdefault(mt, []).append(fn)

                # idx0 (1,0): V1 per-mt (3 m-tiles ahead of its zpv use),
                # k1 ch2..7 paced with the F1 DMA, q2-ch3 for idx1
                for mt in range(MT):
                    add_inj(0, mt, (lambda m=mt: proj_v_mt(1, m)))
                for c_ in range(2, 8):
                    add_inj(0, 4 * (c_ - 1),
                            (lambda cc=c_: proj_qk_pack(kT[1], "k1", F1, cc)))
                add_inj(0, 27, lambda: proj_qk_pack(qT[1], "q2", F2, 3))
                # idx1 (1,3): k2 (for (0,*) scores), q1-ch0
                for c_ in range(8):
                    add_inj(1, 3 * c_,
                            (lambda cc=c_: proj_qk_pack(kT[0], "k2", F2, cc)))
                add_inj(1, 24, lambda: proj_qk_pack(qT[0], "q1", F1, 0))
                # idx2 (0,0): V2 per-mt, q1-ch3
                for mt in range(MT):
                    add_inj(2, mt, (lambda m=mt: proj_v_mt(0, m)))
                add_inj(2, 26, lambda: proj_qk_pack(qT[0], "q1", F1, 3))
                add_inj(3, 14, lambda: proj_qk_pack(qT[1], "q2", F2, 2))
                add_inj(4, 14, lambda: proj_qk_pack(qT[1], "q2", F2, 1))
                add_inj(5, 14, lambda: proj_qk_pack(qT[0], "q1", F1, 2))
                add_inj(6, 14, lambda: proj_qk_pack(qT[0], "q1", F1, 1))

                # pre-attention projections: k1 ch0/ch1, q2 ch0
                proj_qk_pack(kT[1], "k1", F1, 0)
                proj_qk_pack(kT[1], "k1", F1, 1)
                proj_qk_pack(qT[1], "q2", F2, 0)

                pending_epi = None   # (stage1, stage2) of previous chunk
                nlast = len(chunk_seq) - 1
                for idx, (a, ch) in enumerate(chunk_seq):
                    inject = {mt: list(fns)
                              for mt, fns in inject_plan[idx].items()}
                    if pending_epi is not None:
                        inject.setdefault(2, []).insert(0, pending_epi[0])
                        inject.setdefault(8, []).insert(0, pending_epi[1])
                    pv = [pvps.tile([128, CH], f32, name=f"pv{c_}",
                                    tag="pv") for c_ in range(2)]
                    esum = epool.tile([128, CH], f32, name="esum",
                                      tag="esum", bufs=2)
                    epi = make_epilogue(a, ch, pv, esum)
                    tail = epi[0] if idx == nlast else None
                    attn_chunk(a, ch, pv, esum, inject, tail)
                    if idx == nlast:
                        # stage2 right after the flush; conv1's first chunks
                        # (cits 2,3, which don't read these rows) overlap it
                        epi[1]()
                        pending_epi = None
                    else:
                        pending_epi = epi
                    if idx == 2:    # a=1 halo chunks written (idx 0,1)
                        pack_cc0(0)
                    if idx == 4:
                        unpack_cc0(0)
                        pack_cc0(1)   # a=0 halo chunks written (idx 2,3)
                    if idx == 6:
                        unpack_cc0(1)

            fmaps_ctx.__exit__(None, None, None)

            if debug:
                nc.sync.dma_start(out=dbg["qT1"][:], in_=qT[0][:])
                nc.sync.dma_start(out=dbg["kT2"][:], in_=kT[0][:])
                nc.sync.dma_start(out=dbg["V2"][:], in_=Vv[0][:])

            qkv_ctx.__exit__(None, None, None)

            if debug:
                nc.sync.dma_start(out=dbg["X1"][:], in_=X1[:])

            # =========== PHASE 3: conv stack ===========
            Xcur = X1
            with (
                tc.tile_pool(name="cpsum", bufs=8, space="PSUM") as cpsum,
                tc.tile_pool(name="convw", bufs=2) as convw,
            ):
                # read order: pss[3] first (needs only chunks 2,3 + bottom
                # ghost of the previous layer, the earliest-normalized rows)
                CH_ORDER = (3, 2, 1, 0)
                # close order for the last cit: halo chunks (0, 3) close
                # first so the pairwise halo AG fires half a cit early
                CLOSE_ORDER = (0, 3, 2, 1)

                def conv_layer_matmuls(li, cot, wct, cit_list, pss,
                                       start_cits, stop_cits, order,
                                       on_stop=None):
                    """Emit the 9-tap matmuls for the given cits of one cot.
                    start_cits/stop_cits: cit values that carry start/stop.
                    on_stop(ch) emits the psum eviction + local stats right
                    after chunk ch's accumulation group closes."""
                    parts = min(CONVS[li][1], 128)
                    for cit in cit_list:
                        xv = Xcur[:, cit].rearrange("p (r c) -> p r c",
                                                    c=PADW)
                        for ch in order:
                            for dy in range(3):
                                for dx in range(3):
                                    nc.tensor.matmul(
                                        pss[ch][:],
                                        wct[:, cit, dy, dx,
                                            cot * 128:cot * 128 + parts],
                                        xv[:, ch * 8 + dy:ch * 8 + dy + 8,
                                           dx:dx + 64],
                                        start=(cit in start_cits and dy == 0
                                               and dx == 0),
                                        stop=(cit in stop_cits and dy == 2
                                              and dx == 2))
                            if on_stop is not None and cit in stop_cits:
                                on_stop(ch)

                for li, (cin, cout, cit_n, cot_n) in enumerate(CONVS):
                    parts = min(cout, 128)
                    last = li == len(CONVS) - 1
                    wct = wc_sb[li]
                    bnt = bn_sb[li]  # [parts, 3(bc,g,bb), cot_n]
                    if not last:
                        Xnext = xpads.tile([128, cot_n, NPAD], bf16,
                                           name=f"X{li+2}", tag=f"X{li+2}")
                        # only the pad columns (0, 65) need zeroing: ghost
                        # rows are fully written by the halo path
                        for cot in range(cot_n):
                            xnf = Xnext[:, cot]
                            nc.vector.memset(bass.AP(
                                tensor=xnf.tensor, offset=xnf.offset,
                                ap=[list(xnf.ap[0]),
                                    [PADW, PADR], [65, 2]]), 0.0)
                    yf = [convw.tile([parts, NQ], f32, name=f"y{li}_{cot}",
                                     tag=f"y{li}_{cot}", bufs=1)
                          for cot in range(cot_n)]
                    pss_all = []
                    for cot in range(cot_n):
                        pss = {}
                        for ch in CH_ORDER:
                            pss[ch] = cpsum.tile([parts, CH], f32,
                                                 name=f"cps{cot}_{ch}",
                                                 tag="cps")
                        pss_all.append(pss)

                        # evictions (+conv bias) and local BN stats fire
                        # per chunk, as each accumulation group closes;
                        # halo AG fires once chunks 0 and 3 have closed
                        hstg = (stage.tile([parts, 2, 64], f32,
                                           name=f"hstg{li}_{cot}",
                                           tag="hstg")
                                if not last else None)
                        sstg = stage.tile([parts, 2], f32,
                                          name=f"sstg{li}_{cot}", tag="sstg")
                        bns = convw.tile([parts, 4, 6], f32, name="bns",
                                         tag="bns")
                        closed = []

                        def on_stop(ch, cot=cot, hstg=hstg, bns=bns,
                                    closed=closed):
                            nc.scalar.activation(
                                out=yf[cot][:, ch * CH:(ch + 1) * CH],
                                in_=pss[ch][:], func=FT.Identity,
                                bias=bnt[:, 0, cot:cot + 1])
                            # stats straight off the PSUM bank (no wait on
                            # the eviction); the conv bias only shifts the
                            # mean, corrected after the AllGather
                            nc.vector.bn_stats(
                                out=bns[:, ch], in_=pss[ch][:])
                            if not last:
                                if ch == 0:
                                    nc.vector.tensor_copy(
                                        hstg[:, 0], yf[cot][:, 0:64])
                                if ch == 3:
                                    nc.vector.tensor_copy(
                                        hstg[:, 1], yf[cot][:, NQ - 64:NQ])
                                closed.append(ch)
                                if set(closed) >= {0, 3} and \
                                        "fired" not in closed:
                                    closed.append("fired")
                                    nc.sync.dma_start(
                                        out=hli[li][cot][:],
                                        in_=hstg[:])
                                    nc.gpsimd.collective_compute(
                                        "AllGather", mybir.AluOpType.bypass,
                                        ins=[hli[li][cot][:]],
                                        outs=[hlo[li][cot][:]],
                                        replica_groups=rpairs)

                        if li == 1:
                            # conv2: consume conv1-cot0 channels (cit 0) for
                            # all pss chunks first; conv1-cot1's stats
                            # AllReduce hides behind them.
                            conv_layer_matmuls(li, cot, wct, [0], pss,
                                               {0}, set(), CH_ORDER)
                            conv_layer_matmuls(li, cot, wct, [1], pss,
                                               set(), {1}, CLOSE_ORDER,
                                               on_stop=on_stop)
                        elif li == 0:
                            # conv1: a=1 channels (X1 cits 2,3 -- done early
                            # in the attention chunk order) first
                            conv_layer_matmuls(li, cot, wct, [2, 3, 0], pss,
                                               {2}, set(), CH_ORDER)
                            conv_layer_matmuls(li, cot, wct, [1], pss,
                                               set(), {1}, CLOSE_ORDER,
                                               on_stop=on_stop)
                        else:
                            conv_layer_matmuls(li, cot, wct, [0], pss,
                                               {0}, {0}, CH_ORDER,
                                               on_stop=on_stop)

                        mv = convw.tile([parts, 2], f32, name="mv", tag="mv")
                        nc.vector.bn_aggr(out=mv[:], in_=bns[:])
                        # sum = mean*2048 ; sumsq = (var + mean^2)*2048
                        nc.vector.tensor_scalar_mul(
                            sstg[:, 0:1], mv[:, 0:1], float(NQ))
                        m2 = convw.tile([parts, 1], f32, name="m2", tag="m2")
                        nc.vector.tensor_mul(m2[:], mv[:, 0:1], mv[:, 0:1])
                        nc.vector.tensor_add(sstg[:, 1:2], mv[:, 1:2], m2[:])
                        nc.vector.tensor_scalar_mul(
                            sstg[:, 1:2], sstg[:, 1:2], float(NQ))
                        nc.sync.dma_start(out=sli[li][cot][:], in_=sstg[:])
                        # AllGather + receive-side reduce: the 8-rank
                        # AG mesh measures ~4us faster than AllReduce
                        nc.gpsimd.collective_compute(
                            "AllGather", mybir.AluOpType.bypass,
                            ins=[sli[li][cot][:]], outs=[slo[li][cot][:]],
                            replica_groups=replica8)

                        # ---- receive path, emitted per cot so cot0's
                        # normalize overlaps cot1's matmul stream (keeps
                        # sync's DMA queue free of priority inversions:
                        # halo0, stats0, sGL0, halo1, stats1, sGL1) ----
                        if not last:
                            hGL = convw.tile([parts, 2, 128], f32,
                                             name=f"hGL{li}_{cot}",
                                             tag="hGL")
                            hap = hlo[li][cot][:]
                            nc.gpsimd.dma_start(out=hGL[:], in_=bass.AP(
                                tensor=hap.tensor, offset=0,
                                ap=[[128, parts], [parts * 128, 2],
                                    [1, 128]]))
                            hgap = hGL[:]
                        sGL = convw.tile([parts, 8, 2], f32, name="sGL",
                                         tag="sGL")
                        sap = slo[li][cot][:]
                        nc.sync.dma_start(out=sGL[:], in_=bass.AP(
                            tensor=sap.tensor, offset=0,
                            ap=[[2, parts], [parts * 2, 8], [1, 2]]))
                        sgap = sGL[:]
                        ssl = bass.AP(tensor=sgap.tensor, offset=sgap.offset,
                                      ap=[list(sgap.ap[0]), [1, 2], [2, 8]])
                        tot = convw.tile([parts, 2], f32, name="tot",
                                         tag="tot")
                        nc.vector.reduce_sum(tot[:], ssl,
                                             axis=mybir.AxisListType.X)
                        # scale/bias: rstd = (Q/N - (S/N)^2 + eps)^-1/2
                        ms = convw.tile([parts, 2], f32, name="ms", tag="ms")
                        nc.vector.tensor_scalar_mul(ms[:], tot[:],
                                                    1.0 / 16384.0)
                        m2b = convw.tile([parts, 1], f32, name="m2b",
                                         tag="m2b")
                        nc.vector.tensor_mul(m2b[:], ms[:, 0:1], ms[:, 0:1])
                        var = convw.tile([parts, 1], f32, name="var",
                                         tag="var")
                        nc.vector.tensor_sub(var[:], ms[:, 1:2], m2b[:])
                        lnv = convw.tile([parts, 1], f32, name="lnv",
                                         tag="lnv")
                        nc.scalar.activation(out=lnv[:], in_=var[:],
                                             func=FT.Ln, bias=epsb[:parts])
                        rstd = convw.tile([parts, 1], f32, name="rstd",
                                          tag="rstd")
                        nc.scalar.activation(out=rstd[:], in_=lnv[:],
                                             func=FT.Exp, scale=-0.5)
                        scl = convw.tile([parts, 1], f32, name="scl",
                                         tag="scl")
                        nc.vector.tensor_mul(scl[:], bnt[:, 1, cot:cot + 1],
                                             rstd[:])
                        bia = convw.tile([parts, 1], f32, name="bia",
                                         tag="bia")
                        meanY = convw.tile([parts, 1], f32, name="meanY",
                                           tag="meanY")
                        nc.vector.tensor_add(meanY[:], ms[:, 0:1],
                                             bnt[:, 0, cot:cot + 1])
                        nc.vector.tensor_mul(bia[:], meanY[:], scl[:])
                        nc.vector.tensor_sub(bia[:], bnt[:, 2, cot:cot + 1],
                                             bia[:])
                        if debug and li == 0:
                            nc.sync.dma_start(out=dbg["SC1"][cot, :, 0:1],
                                              in_=scl[:])
                            nc.sync.dma_start(out=dbg["SC1"][cot, :, 1:2],
                                              in_=bia[:])

                        if last:
                            # final relu per chunk + output DMA on 4 queues
                            dma_engines = [nc.sync, nc.scalar, nc.gpsimd,
                                           nc.sync]
                            for i, ch in enumerate((0, 1, 2, 3)):
                                outf = convw.tile([parts, CH], f32,
                                                  name=f"outf{ch}",
                                                  tag="outf", bufs=4)
                                nc.scalar.activation(
                                    out=outf[:],
                                    in_=yf[cot][:, ch * CH:(ch + 1) * CH],
                                    func=FT.Relu, scale=scl[:], bias=bia[:])
                                dma_engines[i].dma_start(
                                    out=d_out[:, ch * CH:(ch + 1) * CH],
                                    in_=outf[:])
                        else:
                            xv = Xnext[:, cot].rearrange("p (r c) -> p r c",
                                                         c=PADW)

                            def norm_chunk(ch):
                                nc.scalar.activation(
                                    out=xv[:parts, 1 + ch * 8:9 + ch * 8,
                                           1:65],
                                    in_=yf[cot][:, ch * CH:(ch + 1) * CH]
                                    .rearrange("p (r w) -> p r w", w=64),
                                    func=FT.Relu, scale=scl[:], bias=bia[:])

                            def ghost_row(td, ts_):
                                # ghost rows: 2-slot select from the
                                # pairwise-gathered buffer
                                gsl = bass.AP(
                                    tensor=hgap.tensor,
                                    offset=hgap.offset + ts_ * 64,
                                    ap=[list(hgap.ap[0]), [1, 64],
                                        [128, 2]])
                                prod = convw.tile([parts, 64, 2], f32,
                                                  name="prodL", tag="prodL")
                                nc.vector.tensor_mul(prod[:], gsl,
                                                     selF2[:parts, td])
                                nc.vector.tensor_add(prod[:, :, 0:1],
                                                     prod[:, :, 0:1],
                                                     prod[:, :, 1:2])
                                gb = convw.tile([parts, 64], bf16,
                                                name="gbL", tag="gbL")
                                nc.scalar.activation(out=gb[:],
                                                     in_=prod[:, :, 0],
                                                     func=FT.Relu,
                                                     scale=scl[:], bias=bia[:])
                                pr = 0 if td == 0 else 33
                                nc.vector.tensor_mul(xv[:parts, pr, 1:65],
                                                     gb[:],
                                                     rowm[:parts, td])

                            # order matched to the next layer's CH_ORDER
                            # (3,2,1,0): pss[3] needs chunks 2,3 + bottom
                            # ghost; pss[2] adds chunk 1; pss[0] is last.
                            norm_chunk(3)
                            norm_chunk(2)
                            ghost_row(1, 0)   # bottom ghost (row 33)
                            norm_chunk(1)
                            norm_chunk(0)
                            ghost_row(0, 1)   # top ghost (row 0)
                    if debug and li == 0:
                        for cot in range(cot_n):
                            nc.sync.dma_start(out=dbg["Y1"][cot],
                                              in_=yf[cot][:])
                        if not last:
                            nc.sync.dma_start(out=dbg["X2"][:], in_=Xnext[:])
                    if not last:
                        Xcur = Xnext

    n = _split_excess_waits(nc, 1)
    return nc, n


def _shard_inputs(inputs):
    """Build the 8 per-core input maps from the full problem inputs."""
    bf = ml_dtypes.bfloat16
    fm1 = np.asarray(inputs["feature_map1"], np.float32)
    fm2 = np.asarray(inputs["feature_map2"], np.float32)

    def pshuf(a2d):  # [2*128, X] -> [128, 2*X] partition-major
        n2, x = a2d.shape
        kt = n2 // 128
        return np.ascontiguousarray(
            a2d.reshape(kt, 128, x).transpose(1, 0, 2).reshape(128, kt * x))

    shared = {}
    for nm in ("q1", "k2", "v2", "q2", "k1", "v1"):
        wT = np.asarray(inputs[f"{nm}_w"], np.float32).T  # [in, out]
        shared[f"w_{nm}"] = pshuf(wT).astype(bf)
    for nm in ("q1", "k2", "q2", "k1"):
        b = np.asarray(inputs[f"{nm}_b"], np.float32)
        shared[f"b_{nm}"] = np.ascontiguousarray(b.reshape(2, 128).T)
    shared["bv1r"] = np.tile(np.asarray(inputs["v1_b"], np.float32)[None, :],
                             (128, 1))
    shared["bv2r"] = np.tile(np.asarray(inputs["v2_b"], np.float32)[None, :],
                             (128, 1))
    for li, (cin, cout, cit_n, cot_n) in enumerate(CONVS):
        wc = np.asarray(inputs[f"conv{li+1}_w"], np.float32)  # [co, ci, 3, 3]
        # -> [p, cit, ky, kx, co]
        arr = wc.transpose(1, 2, 3, 0).reshape(cit_n, 128, 3, 3, cout)
        arr = arr.transpose(1, 0, 2, 3, 4).reshape(128, -1)
        shared[f"wc{li}"] = np.ascontiguousarray(arr).astype(bf)
        parts = min(cout, 128)
        cot_nn = cout // parts
        trio = np.stack([
            np.asarray(inputs[f"conv{li+1}_b"], np.float32),
            np.asarray(inputs[f"bn{li+1}_g"], np.float32),
            np.asarray(inputs[f"bn{li+1}_b"], np.float32),
        ])  # [3, cout]
        # -> [parts, 3, cot_n] -> [parts, 3*cot_n]
        arr = trio.reshape(3, cot_nn, parts).transpose(2, 0, 1)
        shared[f"bn{li}"] = np.ascontiguousarray(arr.reshape(parts, -1))

    in_maps = []
    for r in range(N_CORES):
        b, half = divmod(r, 2)
        h0 = 32 * half
        m = dict(shared)
        # roll rows so this core's query rows are columns 0:2048
        m["f1"] = pshuf(np.roll(fm1[b], -h0, axis=1).reshape(C, HW)).astype(bf)
        m["f2"] = pshuf(np.roll(fm2[b], -h0, axis=1).reshape(C, HW)).astype(bf)
        # ghost row selection over the 2 pair slots:
        # dest td=0 (top ghost) / td=1 (bottom ghost)
        sel = np.zeros((2, 2), np.float32)
        pslot = 1 - (r & 1)     # partner's slot within the pair
        if half == 0:
            sel[1, pslot] = 1.0   # bottom ghost <- partner's top row
        else:
            sel[0, pslot] = 1.0   # top ghost <- partner's bottom row
        selfull = np.broadcast_to(sel[None, :, None, :],
                                  (128, 2, 64, 2)).copy()
        m["sel2"] = selfull.astype(bf)
        m["selF2"] = selfull.astype(np.float32)
        rowmask = sel.sum(-1)  # [2]
        m["rowm"] = np.broadcast_to(rowmask[None, :, None],
                                    (128, 2, 64)).copy().astype(bf)
        in_maps.append(m)
    return in_maps


def _get_program(debug=False):
    key = ("dbg" if debug else "rel")
    if key not in _CACHE:
        _CACHE[key] = _build_program(debug=debug)
    return _CACHE[key]


def run(inputs, trace=False, debug=False):
    from concourse.bass_utils import run_bass_kernel_spmd
    nc, _ = _get_program(debug=debug)
    in_maps = _shard_inputs(inputs)
    res = run_bass_kernel_spmd(nc, in_maps, list(range(N_CORES)), trace=trace)
    out = np.zeros((B, 64, H, W), np.float32)
    for r in range(N_CORES):
        b, half = divmod(r, 2)
        h0 = 32 * half
        out[b, :, h0:h0 + 32, :] = res.results[r]["yout"].reshape(64, 32, 64)
    return out, res


def kernel(**inputs):
    out, _ = run(inputs, trace=False)
    return out
